# revision 1
# baseline (speedup 1.0000x reference)
"""Bucket (block-diagonal) attention layer for Trainium2, 8 NeuronCores SPMD.

Sharding: data-parallel over batch (4) x tensor-parallel over head groups (2).
Core c = b*2 + g handles batch b, global heads [g*8, g*8+8).

Per-core math (local out dim 512 = 8 heads x 64):
  qT[dl, t] = sum_k Wq[g*512+dl, k] * x[b, t, k]  (+ bq)   [transposed layout]
  kT[dl, t] = likewise (bk dropped: constant-per-row score shifts cancel in
              softmax -- only bq enters scores via bq . k_j)
  v[t, dl]  = natural layout, with a ones-column appended per head so the
              attended matmul also produces the softmax denominator.
  scoresT[kt, qt] = matmul(lhsT=kT_head, rhs=qT_head)      (K=64)
  expT = exp(scoresT)  (no max subtraction; logits sigma ~3.3, safe in f32)
  att[qt, 0:64], den[qt] = matmul(lhsT=expT, rhs=[v_head | ones])
  y = att / den + (x_slice + bv)   [residual + bv folded on host]

All matmuls bf16 (f32 accumulate in PSUM); softmax/normalize in f32.
"""

import json
import sys

import numpy as np
import ml_dtypes

BF16 = ml_dtypes.bfloat16
FP16 = np.float16

B, S, D = 4, 4096, 1024
H, NB = 16, 32
HG = 2            # head groups (tensor parallel over heads)
NCORES = B * HG   # 8
DL = D // HG      # 512 local output dims per core
HL = H // HG      # 8 local heads
HD = D // H       # 64 head dim
BS = S // NB      # 128 bucket size
KC = D // 128     # 8 contraction chunks
NQ = 4            # token quarters processed as pipeline phases
TOKQ = S // NQ    # 1024 tokens per quarter
NBQ = TOKQ // BS  # 8 buckets per quarter
VW = 66           # per-head block width in v tiles: 64 data + 1 ones + 1 pad

_built = None     # cached (nc,) so repeated kernel() calls reuse the program


def _apply_waitfix():
    """This container's walrus accepts at most ONE sem wait per instruction.
    Post-process the BIR json: hoist extra waits onto injected wait-only
    EventSemaphore instructions just before the owning instruction."""
    import concourse.bass as bass

    if getattr(bass.Bass, "_waitfix_applied", False):
        return
    orig = bass.Bass.to_json_bytes

    def _split(m):
        n = 0
        for f in m["functions"]:
            for blk in f["blocks"]:
                out = []
                for inst in blk["instructions"]:
                    si = inst.get("sync_info")
                    if si and si.get("on_wait") and len(si["on_wait"]) > 1:
                        waits = si["on_wait"]
                        si["on_wait"] = waits[-1:]
                        for k, w in enumerate(waits[:-1]):
                            out.append({
                                "debug": inst.get("debug", 0),
                                "engine": inst["engine"],
                                "ins": [],
                                "outs": [],
                                "name": f"wfix{n}_{k}_{inst['name']}",
                                "opcode": "EventSemaphore",
                                "sync_info": {"on_update": [], "on_wait": [w]},
                            })
                        n += 1
                    out.append(inst)
                blk["instructions"] = out
        return n

    def patched(self):
        m = json.loads(orig(self))
        _split(m)
        return json.dumps(m).encode()

    bass.Bass.to_json_bytes = patched
    bass.Bass._waitfix_applied = True


def _build():
    global _built
    if _built is not None:
        return _built

    _apply_waitfix()
    import concourse.bass as bass
    import concourse.tile as tile
    from concourse import mybir
    from concourse.bass import ts

    f32 = mybir.dt.float32
    bf16 = mybir.dt.float16
    Act = mybir.ActivationFunctionType
    Alu = mybir.AluOpType

    nc = bass.Bass()
    xt = nc.dram_tensor("xt", [D, S], bf16, kind="ExternalInput")
    wq = nc.dram_tensor("wq", [D, DL], bf16, kind="ExternalInput")
    wk = nc.dram_tensor("wk", [D, DL], bf16, kind="ExternalInput")
    wv = nc.dram_tensor("wv", [D, DL], bf16, kind="ExternalInput")
    bqt = nc.dram_tensor("bq", [128, DL // 128], f32, kind="ExternalInput")
    xres = nc.dram_tensor("xres", [S, DL], f32, kind="ExternalInput")
    y = nc.dram_tensor("y", [S, DL], f32, kind="ExternalOutput")

    OD = DL // 128  # 4 out-dim partition tiles for qT/kT

    with tile.TileContext(nc) as tc:
        with (
            tc.tile_pool(name="wpool", bufs=1) as wpool,
            tc.tile_pool(name="xtp", bufs=12) as xtp,
            tc.tile_pool(name="qtp", bufs=2 * OD) as qtp,
            tc.tile_pool(name="ktp", bufs=2 * OD) as ktp,
            tc.tile_pool(name="vp", bufs=2 * NBQ) as vpool,
            tc.tile_pool(name="ep", bufs=6) as epool,
            tc.tile_pool(name="yp", bufs=3) as ypool,
            tc.tile_pool(name="xrp", bufs=4) as xrpool,
            tc.tile_pool(name="rp", bufs=8) as rpool,
            # HW constraint found empirically: each start=True matmul group
            # needs its OWN psum bank (same-bank groups corrupt for K<128 and
            # crash for mixed base partitions). 2 + 4 + 2 = 8 banks.
            tc.tile_pool(name="ps_qkv", bufs=2, space="PSUM") as ps_qkv,
            tc.tile_pool(name="ps_s", bufs=4, space="PSUM") as ps_s,
            tc.tile_pool(name="ps_a", bufs=2, space="PSUM") as ps_a,
        ):
            # --- stationary weights + bias, loaded once ---
            # wq/wk first: they gate the first projection matmuls; wv only
            # gates the v phase which runs later.
            wq_sb, wk_sb, wv_sb = [], [], []
            for lst, src, nm in ((wq_sb, wq, "wq"), (wk_sb, wk, "wk"),
                                 (wv_sb, wv, "wv")):
                for kk in range(KC):
                    t = wpool.tile([128, DL], bf16, tag=f"{nm}{kk}",
                                   name=f"{nm}{kk}")
                    nc.sync.dma_start(out=t[:], in_=src[ts(kk, 128), :])
                    lst.append(t)
            bq_sb = wpool.tile([128, OD], f32, tag="bq")
            nc.sync.dma_start(out=bq_sb[:], in_=bqt[:, :])

            for q in range(NQ):
                tok0 = q * TOKQ
                # --- load xT chunks for this quarter ---
                xt_sb = []
                for kk in range(KC):
                    t = xtp.tile([128, TOKQ], bf16, tag="xt")
                    nc.sync.dma_start(
                        out=t[:], in_=xt[ts(kk, 128), tok0:tok0 + TOKQ])
                    xt_sb.append(t)

                # --- q/k projections: psum[od-tile, 512 tok] over 8 k-chunks
                qt_sb = [qtp.tile([128, TOKQ], bf16, tag="qt", name="qt")
                         for _ in range(OD)]
                kt_sb = [ktp.tile([128, TOKQ], bf16, tag="kt", name="kt")
                         for _ in range(OD)]
                for od in range(OD):
                    for tt in range(TOKQ // 512):
                        pq = ps_qkv.tile([128, 512], f32, tag="pqkv")
                        for kk in range(KC):
                            nc.tensor.matmul(
                                pq[:], wq_sb[kk][:, ts(od, 128)],
                                xt_sb[kk][:, ts(tt, 512)],
                                start=(kk == 0), stop=(kk == KC - 1))
                        nc.scalar.activation(
                            qt_sb[od][:, ts(tt, 512)], pq[:], Act.Identity,
                            bias=bq_sb[:, od:od + 1], scale=1.0)
                        pk = ps_qkv.tile([128, 512], f32, tag="pqkv")
                        for kk in range(KC):
                            nc.tensor.matmul(
                                pk[:], wk_sb[kk][:, ts(od, 128)],
                                xt_sb[kk][:, ts(tt, 512)],
                                start=(kk == 0), stop=(kk == KC - 1))
                        nc.scalar.copy(kt_sb[od][:, ts(tt, 512)], pk[:])

                # --- v projection (natural layout), one bucket per psum ---
                v_sb = []
                for vt in range(NBQ):
                    pv = ps_qkv.tile([128, 512], f32, tag="pqkv")
                    for kk in range(KC):
                        nc.tensor.matmul(
                            pv[:], xt_sb[kk][:, ts(vt, 128)], wv_sb[kk][:],
                            start=(kk == 0), stop=(kk == KC - 1))
                    vt_sb = vpool.tile([128, HL * VW], f32, tag="v")
                    v3 = vt_sb[:].rearrange("p (h c) -> p h c", c=VW)
                    nc.vector.memset(v3[:, :, 64:66], 1.0)
                    nc.vector.tensor_copy(
                        v3[:, :, 0:64],
                        pv[:].rearrange("p (h c) -> p h c", c=HD))
                    v_sb.append(vt_sb)

                # --- attention per bucket ---
                for bk in range(NBQ):
                    col = ts(bk, BS)  # token slice within quarter
                    xr = xrpool.tile([128, DL], f32, tag="xres")
                    nc.sync.dma_start(
                        out=xr[:], in_=xres[tok0 + bk * BS:tok0 + (bk + 1) * BS, :])
                    yt = ypool.tile([128, DL], f32, tag="yt")
                    for h in range(HL):
                        od, po = h // 2, (h % 2) * 64
                        psc = ps_s.tile([128, 128], f32, tag="ps", name="ps")
                        nc.tensor.matmul(
                            psc[:],
                            kt_sb[od][po:po + 64, col],
                            qt_sb[od][po:po + 64, col],
                            start=True, stop=True)
                        ex = epool.tile([128, 128], f32, tag="expT",
                                        name="ex")
                        nc.scalar.activation(ex[:], psc[:], Act.Exp)
                        pa = ps_a.tile([128, VW], f32, tag="pa", name="pa")
                        nc.tensor.matmul(
                            pa[:], ex[:],
                            v_sb[bk][:, h * VW:(h + 1) * VW],
                            start=True, stop=True)
                        rc = rpool.tile([128, 1], f32, tag="r", name="rc")
                        nc.vector.reciprocal(rc[:], pa[:, 64:65])
                        nc.vector.scalar_tensor_tensor(
                            out=yt[:, ts(h, HD)],
                            in0=pa[:, 0:64],
                            scalar=rc[:],
                            in1=xr[:, ts(h, HD)],
                            op0=Alu.mult, op1=Alu.add)
                    nc.sync.dma_start(
                        out=y[tok0 + bk * BS:tok0 + (bk + 1) * BS, :], in_=yt[:])

    _built = nc
    return nc


def _prep_in_maps(x, Wq, bq, Wk, bk, Wv, bv):
    x = np.asarray(x, np.float32)
    Wq = np.asarray(Wq, np.float32)
    Wv = np.asarray(Wv, np.float32)
    Wk = np.asarray(Wk, np.float32)
    bq = np.asarray(bq, np.float32)
    bv = np.asarray(bv, np.float32)

    xt_b = [np.ascontiguousarray(x[b].T).astype(FP16) for b in range(B)]
    wq_g, wk_g, wv_g, bq_g = [], [], [], []
    for g in range(HG):
        sl = slice(g * DL, (g + 1) * DL)
        wq_g.append(np.ascontiguousarray(Wq[sl, :].T).astype(FP16))
        wk_g.append(np.ascontiguousarray(Wk[sl, :].T).astype(FP16))
        wv_g.append(np.ascontiguousarray(Wv[sl, :].T).astype(FP16))
        bq_g.append(np.ascontiguousarray(
            bq[sl].reshape(DL // 128, 128).T).astype(np.float32))

    in_maps = []
    for c in range(NCORES):
        b, g = c // HG, c % HG
        sl = slice(g * DL, (g + 1) * DL)
        xres = (x[b][:, sl] + bv[None, sl]).astype(np.float32)
        in_maps.append({
            "xt": xt_b[b], "wq": wq_g[g], "wk": wk_g[g], "wv": wv_g[g],
            "bq": bq_g[g], "xres": np.ascontiguousarray(xres),
        })
    return in_maps


def _gather(results):
    out = np.empty((B, S, D), np.float32)
    for c, r in enumerate(results):
        b, g = c // HG, c % HG
        out[b, :, g * DL:(g + 1) * DL] = r["y"]
    return out


def _run(inputs, trace=False, trace_cores=None):
    nc = _build()
    from concourse.bass_utils import run_bass_kernel_spmd

    in_maps = _prep_in_maps(**inputs)
    res = run_bass_kernel_spmd(
        nc, in_maps, core_ids=list(range(NCORES)), trace=trace,
        trace_cores=trace_cores)
    return _gather(res.results), res


def kernel(**inputs):
    out, _ = _run(inputs, trace=False)
    return out


def kernel_traced(trace_cores=None, **inputs):
    """For test.py: returns (output, BassKernelResults with exec_time_ns)."""
    import types
    import trn_agent_boot.trn_boot as tb

    if "antenv.axon_hooks" not in sys.modules:
        hooks = types.ModuleType("antenv.axon_hooks")
        state = [None]
        hooks.set_axon_ntff_profile_hook = lambda h: state.__setitem__(0, h)
        hooks.get_axon_ntff_profile_hook = lambda: state[0]
        sys.modules["antenv.axon_hooks"] = hooks
        hooks.set_axon_ntff_profile_hook(
            tb._ntff_profile_via_ctypes("/opt/axon/libaxon_pjrt.so"))
    return _run(inputs, trace=True, trace_cores=trace_cores)



# revision 2
# speedup vs baseline: 1.5238x; 1.5238x over previous
"""Bucket (block-diagonal) attention layer for Trainium2, 8 NeuronCores SPMD.

Sharding: data-parallel over batch (4) x tensor-parallel over head groups (2).
Core c = b*2 + g handles batch b, global heads [g*8, g*8+8).

Per-core math (local out dim 512 = 8 heads x 64):
  qT[dl, t] = sum_k Wq[g*512+dl, k] * x[b, t, k]  (+ bq)   [transposed layout]
  kT[dl, t] = likewise (bk dropped: constant-per-row score shifts cancel in
              softmax -- only bq enters scores via bq . k_j)
  v[t, dl]  = natural layout (bf16), with a ones-column appended per head so
              the attended matmul also produces the softmax denominator.
  scoresT[kt, qt] = matmul(lhsT=kT_head, rhs=qT_head)      (K=64)
  expT = exp(scoresT) in bf16 (no max subtraction; logits sigma ~3.3)
  att[qt, 0:64], den[qt] = matmul(lhsT=expT, rhs=[v_head | ones])  (bf16)
  y = att / den + (x_slice + bv)   [residual + bv folded on host, fp16]

Perf structure vs v1 baseline:
 - all attention matmuls 16-bit (v1 ran them fp32 = 4 cycles/row on PE)
 - scores for 4 heads share one PSUM bank -> one batched EXP per [128,512]
 - attended for 4 heads share one bank -> batched reciprocal + strided
   broadcast normalize on DVE (v1: per-head ops)
 - projections of quarter q+1 are emission-interleaved with attention of
   quarter q so the PE stays dense (HAM stays warm) and softmax latency
   hides under projection matmuls.
"""

import json
import sys

import numpy as np

FP16 = np.float16

B, S, D = 4, 4096, 1024
H, NB = 16, 32
HG = 2            # head groups (tensor parallel over heads)
NCORES = B * HG   # 8
DL = D // HG      # 512 local output dims per core
HL = H // HG      # 8 local heads
HD = D // H       # 64 head dim
BS = S // NB      # 128 bucket size
KC = D // 128     # 8 contraction chunks
NQ = 4            # token quarters processed as pipeline phases
TOKQ = S // NQ    # 1024 tokens per quarter
NBQ = TOKQ // BS  # 8 buckets per quarter
OD = DL // 128    # 4 out-dim partition tiles for qT/kT
VW = 66           # per-head block width in v tiles: 64 data + 1 ones + 1 pad

_built = None     # cached (nc,) so repeated kernel() calls reuse the program


def _apply_waitfix():
    """This container's walrus accepts at most ONE sem wait per instruction.
    Post-process the BIR json: hoist extra waits onto injected wait-only
    EventSemaphore instructions just before the owning instruction."""
    import concourse.bass as bass

    if getattr(bass.Bass, "_waitfix_applied", False):
        return
    orig = bass.Bass.to_json_bytes

    def _split(m):
        n = 0
        for f in m["functions"]:
            for blk in f["blocks"]:
                out = []
                for inst in blk["instructions"]:
                    si = inst.get("sync_info")
                    if si and si.get("on_wait") and len(si["on_wait"]) > 1:
                        waits = si["on_wait"]
                        si["on_wait"] = waits[-1:]
                        for k, w in enumerate(waits[:-1]):
                            out.append({
                                "debug": inst.get("debug", 0),
                                "engine": inst["engine"],
                                "ins": [],
                                "outs": [],
                                "name": f"wfix{n}_{k}_{inst['name']}",
                                "opcode": "EventSemaphore",
                                "sync_info": {"on_update": [], "on_wait": [w]},
                            })
                        n += 1
                    out.append(inst)
                blk["instructions"] = out
        return n

    def patched(self):
        m = json.loads(orig(self))
        _split(m)
        return json.dumps(m).encode()

    bass.Bass.to_json_bytes = patched
    bass.Bass._waitfix_applied = True


def _build():
    global _built
    if _built is not None:
        return _built

    _apply_waitfix()
    import concourse.bass as bass
    import concourse.tile as tile
    from concourse import mybir
    from concourse.bass import ts

    f32 = mybir.dt.float32
    fp16 = mybir.dt.float16
    bf16 = mybir.dt.bfloat16
    Act = mybir.ActivationFunctionType
    Alu = mybir.AluOpType

    nc = bass.Bass()
    xt = nc.dram_tensor("xt", [D, S], fp16, kind="ExternalInput")
    wq = nc.dram_tensor("wq", [D, DL], fp16, kind="ExternalInput")
    wk = nc.dram_tensor("wk", [D, DL], fp16, kind="ExternalInput")
    wv = nc.dram_tensor("wv", [D, DL], fp16, kind="ExternalInput")
    bqt = nc.dram_tensor("bq", [128, OD], f32, kind="ExternalInput")
    xres = nc.dram_tensor("xres", [S, DL], fp16, kind="ExternalInput")
    y = nc.dram_tensor("y", [S, DL], f32, kind="ExternalOutput")

    with tile.TileContext(nc) as tc:
        with (
            tc.tile_pool(name="wpool", bufs=1) as wpool,
            tc.tile_pool(name="xtp", bufs=16) as xtp,
            tc.tile_pool(name="qtp", bufs=2 * OD) as qtp,
            tc.tile_pool(name="ktp", bufs=2 * OD) as ktp,
            tc.tile_pool(name="vp", bufs=2 * NBQ) as vpool,
            tc.tile_pool(name="ep", bufs=4) as epool,
            tc.tile_pool(name="yp", bufs=3) as ypool,
            tc.tile_pool(name="xrp", bufs=4) as xrpool,
            tc.tile_pool(name="rp", bufs=8) as rpool,
            tc.tile_pool(name="ps_p", bufs=3, space="PSUM") as ps_p,
            tc.tile_pool(name="ps_s", bufs=3, space="PSUM") as ps_s,
            tc.tile_pool(name="ps_a", bufs=2, space="PSUM") as ps_a,
        ):
            # --- stationary weights + bias, loaded once ---
            wq_sb, wk_sb, wv_sb = [], [], []
            for lst, src, nm in ((wq_sb, wq, "wq"), (wk_sb, wk, "wk"),
                                 (wv_sb, wv, "wv")):
                for kk in range(KC):
                    t = wpool.tile([128, DL], fp16, tag=f"{nm}{kk}",
                                   name=f"{nm}{kk}")
                    nc.sync.dma_start(out=t[:], in_=src[ts(kk, 128), :])
                    lst.append(t)
            bq_sb = wpool.tile([128, OD], f32, tag="bq")
            nc.sync.dma_start(out=bq_sb[:], in_=bqt[:, :])

            state = {}  # per-quarter tiles: qt, kt, v

            def load_xt(q):
                tok0 = q * TOKQ
                xt_sb = []
                for kk in range(KC):
                    t = xtp.tile([128, TOKQ], fp16, tag="xt")
                    nc.sync.dma_start(
                        out=t[:], in_=xt[ts(kk, 128), tok0:tok0 + TOKQ])
                    xt_sb.append(t)
                state[q] = {"xt": xt_sb}

            def proj_units(q):
                """Yield 24 emission units: 16 q/k groups + 8 v groups."""
                st = state[q]
                xt_sb = st["xt"]
                qt_sb = [qtp.tile([128, TOKQ], fp16, tag="qt", name="qt")
                         for _ in range(OD)]
                kt_sb = [ktp.tile([128, TOKQ], fp16, tag="kt", name="kt")
                         for _ in range(OD)]
                v_sb = [vpool.tile([128, HL * VW], bf16, tag="v", name="v")
                        for _ in range(NBQ)]
                st["qt"], st["kt"], st["v"] = qt_sb, kt_sb, v_sb

                def qk_unit(which, od, tt):
                    def emit():
                        w_sb = wq_sb if which == "q" else wk_sb
                        dst = qt_sb if which == "q" else kt_sb
                        p = ps_p.tile([128, 512], f32, tag="pp", name="pp")
                        for kk in range(KC):
                            nc.tensor.matmul(
                                p[:], w_sb[kk][:, ts(od, 128)],
                                xt_sb[kk][:, ts(tt, 512)],
                                start=(kk == 0), stop=(kk == KC - 1))
                        if which == "q":
                            nc.scalar.activation(
                                dst[od][:, ts(tt, 512)], p[:], Act.Identity,
                                bias=bq_sb[:, od:od + 1], scale=1.0)
                        else:
                            nc.scalar.copy(dst[od][:, ts(tt, 512)], p[:])
                    return emit

                def v_unit(vt):
                    def emit():
                        p = ps_p.tile([128, 512], f32, tag="pp", name="pp")
                        for kk in range(KC):
                            nc.tensor.matmul(
                                p[:], xt_sb[kk][:, ts(vt, 128)], wv_sb[kk][:],
                                start=(kk == 0), stop=(kk == KC - 1))
                        vt_sb = v_sb[vt]
                        v3 = vt_sb[:].rearrange("p (h c) -> p h c", c=VW)
                        nc.vector.memset(v3[:, :, 64:66], 1.0)
                        nc.vector.tensor_copy(
                            v3[:, :, 0:64],
                            p[:].rearrange("p (h c) -> p h c", c=HD))
                    return emit

                units = []
                for od in range(OD):
                    units.append(qk_unit("q", od, 0))
                    units.append(qk_unit("q", od, 1))
                    units.append(qk_unit("k", od, 0))
                    units.append(qk_unit("k", od, 1))
                for vt in range(NBQ):
                    units.append(v_unit(vt))
                return units

            def attn_bucket(q, bk):
                """Emit one bucket of attention for quarter q."""
                st = state[q]
                qt_sb, kt_sb, v_sb = st["qt"], st["kt"], st["v"]
                tok0 = q * TOKQ
                col = ts(bk, BS)  # token slice within quarter
                xr = xrpool.tile([128, DL], fp16, tag="xres")
                nc.sync.dma_start(
                    out=xr[:],
                    in_=xres[tok0 + bk * BS:tok0 + (bk + 1) * BS, :])
                # scores: even heads -> bank se (4 x 128 cols), odd -> so
                se = ps_s.tile([128, 512], f32, tag="ps", name="ps_e")
                so = ps_s.tile([128, 512], f32, tag="ps", name="ps_o")
                for h in range(HL):
                    od, po = h // 2, (h % 2) * 64
                    bank = se if h % 2 == 0 else so
                    nc.tensor.matmul(
                        bank[:, ts(h // 2, 128)],
                        kt_sb[od][po:po + 64, col],
                        qt_sb[od][po:po + 64, col],
                        start=True, stop=True)
                ex_e = epool.tile([128, 512], bf16, tag="ex", name="ex_e")
                ex_o = epool.tile([128, 512], bf16, tag="ex", name="ex_o")
                nc.scalar.activation(ex_e[:], se[:], Act.Exp)
                nc.scalar.activation(ex_o[:], so[:], Act.Exp)
                # attended: even heads -> bank pe, odd -> po_
                pe = ps_a.tile([128, HL // 2 * VW], f32, tag="pa", name="pa_e")
                po_ = ps_a.tile([128, HL // 2 * VW], f32, tag="pa", name="pa_o")
                for h in range(HL):
                    ex = ex_e if h % 2 == 0 else ex_o
                    bank = pe if h % 2 == 0 else po_
                    slot = h // 2
                    nc.tensor.matmul(
                        bank[:, slot * VW:slot * VW + VW],
                        ex[:, ts(slot, 128)],
                        v_sb[bk][:, h * VW:(h + 1) * VW],
                        start=True, stop=True)
                yt = ypool.tile([128, DL], f32, tag="yt")
                for par, bank in ((0, pe), (1, po_)):
                    pav = bank[:].rearrange("p (h c) -> p h c", c=VW)
                    rc = rpool.tile([128, HL // 2], f32, tag="rc")
                    nc.vector.reciprocal(
                        rc[:].unsqueeze(2), pav[:, :, 64:65])
                    ytv = yt[:].rearrange(
                        "p (h two c) -> p h two c", two=2, c=HD)[:, :, par, :]
                    rcb = rc[:].unsqueeze(2).broadcast_to((128, HL // 2, HD))
                    nc.vector.tensor_tensor(
                        out=ytv, in0=pav[:, :, 0:HD], in1=rcb, op=Alu.mult)
                nc.vector.tensor_tensor(
                    out=yt[:], in0=yt[:], in1=xr[:], op=Alu.add)
                nc.sync.dma_start(
                    out=y[tok0 + bk * BS:tok0 + (bk + 1) * BS, :], in_=yt[:])

            # --- emission: proj(q) interleaved with attention(q-1) ---
            load_xt(0)
            for q in range(NQ):
                if q + 1 < NQ:
                    load_xt(q + 1)
                units = proj_units(q)
                if q == 0:
                    for u in units:
                        u()
                else:
                    ui = 0
                    for bk in range(NBQ):
                        for _ in range(3):
                            if ui < len(units):
                                units[ui]()
                                ui += 1
                        attn_bucket(q - 1, bk)
                    while ui < len(units):
                        units[ui]()
                        ui += 1
                    del state[q - 1]
            for bk in range(NBQ):
                attn_bucket(NQ - 1, bk)

    _built = nc
    return nc


def _prep_in_maps(x, Wq, bq, Wk, bk, Wv, bv):
    x = np.asarray(x, np.float32)
    Wq = np.asarray(Wq, np.float32)
    Wv = np.asarray(Wv, np.float32)
    Wk = np.asarray(Wk, np.float32)
    bq = np.asarray(bq, np.float32)
    bv = np.asarray(bv, np.float32)

    xt_b = [np.ascontiguousarray(x[b].T).astype(FP16) for b in range(B)]
    wq_g, wk_g, wv_g, bq_g = [], [], [], []
    for g in range(HG):
        sl = slice(g * DL, (g + 1) * DL)
        wq_g.append(np.ascontiguousarray(Wq[sl, :].T).astype(FP16))
        wk_g.append(np.ascontiguousarray(Wk[sl, :].T).astype(FP16))
        wv_g.append(np.ascontiguousarray(Wv[sl, :].T).astype(FP16))
        bq_g.append(np.ascontiguousarray(
            bq[sl].reshape(DL // 128, 128).T).astype(np.float32))

    in_maps = []
    for c in range(NCORES):
        b, g = c // HG, c % HG
        sl = slice(g * DL, (g + 1) * DL)
        xres = (x[b][:, sl] + bv[None, sl]).astype(FP16)
        in_maps.append({
            "xt": xt_b[b], "wq": wq_g[g], "wk": wk_g[g], "wv": wv_g[g],
            "bq": bq_g[g], "xres": np.ascontiguousarray(xres),
        })
    return in_maps


def _gather(results):
    out = np.empty((B, S, D), np.float32)
    for c, r in enumerate(results):
        b, g = c // HG, c % HG
        out[b, :, g * DL:(g + 1) * DL] = r["y"]
    return out


def _run(inputs, trace=False, trace_cores=None):
    nc = _build()
    from concourse.bass_utils import run_bass_kernel_spmd

    in_maps = _prep_in_maps(**inputs)
    res = run_bass_kernel_spmd(
        nc, in_maps, core_ids=list(range(NCORES)), trace=trace,
        trace_cores=trace_cores)
    return _gather(res.results), res


def kernel(**inputs):
    out, _ = _run(inputs, trace=False)
    return out


def kernel_traced(trace_cores=None, **inputs):
    """For test.py: returns (output, BassKernelResults with exec_time_ns)."""
    import types
    import trn_agent_boot.trn_boot as tb

    if "antenv.axon_hooks" not in sys.modules:
        hooks = types.ModuleType("antenv.axon_hooks")
        state = [None]
        hooks.set_axon_ntff_profile_hook = lambda h: state.__setitem__(0, h)
        hooks.get_axon_ntff_profile_hook = lambda: state[0]
        sys.modules["antenv.axon_hooks"] = hooks
        hooks.set_axon_ntff_profile_hook(
            tb._ntff_profile_via_ctypes("/opt/axon/libaxon_pjrt.so"))
    return _run(inputs, trace=True, trace_cores=trace_cores)


# revision 4
# speedup vs baseline: 1.5247x; 1.0006x over previous
"""Bucket (block-diagonal) attention layer for Trainium2, 8 NeuronCores SPMD.

Sharding: data-parallel over batch (4) x tensor-parallel over head groups (2).
Core c = b*2 + g handles batch b, global heads [g*8, g*8+8).

Per-core math (local out dim 512 = 8 heads x 64):
  qT[dl, t] = sum_k Wq[g*512+dl, k] * x[b, t, k]  (+ bq)   [transposed layout]
  kT[dl, t] = likewise (bk dropped: constant-per-row score shifts cancel in
              softmax -- only bq enters scores via bq . k_j)
  v[t, dl]  = natural layout (bf16), with a ones-column appended per head so
              the attended matmul also produces the softmax denominator.
  scoresT[kt, qt] = matmul(lhsT=kT_head, rhs=qT_head)      (K=64)
  expT = exp(scoresT) in bf16 (no max subtraction; logits sigma ~3.3)
  att[qt, 0:64], den[qt] = matmul(lhsT=expT, rhs=[v_head | ones])  (bf16)
  y = att / den + (x_slice + bv)   [residual + bv folded on host, fp16]

Perf structure vs v1 baseline:
 - all attention matmuls 16-bit (v1 ran them fp32 = 4 cycles/row on PE)
 - scores for 4 heads share one PSUM bank -> one batched EXP per [128,512]
 - attended for 4 heads share one bank -> batched reciprocal + strided
   broadcast normalize on DVE (v1: per-head ops)
 - projections of quarter q+1 are emission-interleaved with attention of
   quarter q so the PE stays dense (HAM stays warm) and softmax latency
   hides under projection matmuls.
"""

import json
import sys

import numpy as np

FP16 = np.float16

B, S, D = 4, 4096, 1024
H, NB = 16, 32
HG = 2            # head groups (tensor parallel over heads)
NCORES = B * HG   # 8
DL = D // HG      # 512 local output dims per core
HL = H // HG      # 8 local heads
HD = D // H       # 64 head dim
BS = S // NB      # 128 bucket size
KC = D // 128     # 8 contraction chunks
NQ = 4            # token quarters processed as pipeline phases
TOKQ = S // NQ    # 1024 tokens per quarter
NBQ = TOKQ // BS  # 8 buckets per quarter
OD = DL // 128    # 4 out-dim partition tiles for qT/kT
VW = 66           # per-head block width in v tiles: 64 data + 1 ones + 1 pad

_built = None     # cached (nc,) so repeated kernel() calls reuse the program


def _apply_waitfix():
    """This container's walrus accepts at most ONE sem wait per instruction.
    Post-process the BIR json: hoist extra waits onto injected wait-only
    EventSemaphore instructions just before the owning instruction."""
    import concourse.bass as bass

    if getattr(bass.Bass, "_waitfix_applied", False):
        return
    orig = bass.Bass.to_json_bytes

    def _split(m):
        n = 0
        for f in m["functions"]:
            for blk in f["blocks"]:
                out = []
                for inst in blk["instructions"]:
                    si = inst.get("sync_info")
                    if si and si.get("on_wait") and len(si["on_wait"]) > 1:
                        waits = si["on_wait"]
                        si["on_wait"] = waits[-1:]
                        for k, w in enumerate(waits[:-1]):
                            out.append({
                                "debug": inst.get("debug", 0),
                                "engine": inst["engine"],
                                "ins": [],
                                "outs": [],
                                "name": f"wfix{n}_{k}_{inst['name']}",
                                "opcode": "EventSemaphore",
                                "sync_info": {"on_update": [], "on_wait": [w]},
                            })
                        n += 1
                    out.append(inst)
                blk["instructions"] = out
        return n

    def patched(self):
        m = json.loads(orig(self))
        _split(m)
        return json.dumps(m).encode()

    bass.Bass.to_json_bytes = patched
    bass.Bass._waitfix_applied = True


def _build():
    global _built
    if _built is not None:
        return _built

    _apply_waitfix()
    import concourse.bass as bass
    import concourse.tile as tile
    from concourse import mybir
    from concourse.bass import ts

    f32 = mybir.dt.float32
    fp16 = mybir.dt.float16
    bf16 = mybir.dt.bfloat16
    Act = mybir.ActivationFunctionType
    Alu = mybir.AluOpType

    nc = bass.Bass()
    xt = nc.dram_tensor("xt", [D, S], fp16, kind="ExternalInput")
    wq = nc.dram_tensor("wq", [D, DL], fp16, kind="ExternalInput")
    wk = nc.dram_tensor("wk", [D, DL], fp16, kind="ExternalInput")
    wv = nc.dram_tensor("wv", [D, DL], fp16, kind="ExternalInput")
    bqt = nc.dram_tensor("bq", [128, OD], f32, kind="ExternalInput")
    xres = nc.dram_tensor("xres", [S, DL], fp16, kind="ExternalInput")
    y = nc.dram_tensor("y", [S, DL], f32, kind="ExternalOutput")

    with tile.TileContext(nc) as tc:
        with (
            tc.tile_pool(name="wpool", bufs=1) as wpool,
            tc.tile_pool(name="xtp", bufs=16) as xtp,
            tc.tile_pool(name="qtp", bufs=2 * OD) as qtp,
            tc.tile_pool(name="ktp", bufs=2 * OD) as ktp,
            tc.tile_pool(name="vp", bufs=2 * NBQ) as vpool,
            tc.tile_pool(name="ep", bufs=4) as epool,
            tc.tile_pool(name="yp", bufs=3) as ypool,
            tc.tile_pool(name="xrp", bufs=4) as xrpool,
            tc.tile_pool(name="rp", bufs=8) as rpool,
            tc.tile_pool(name="ps_p", bufs=2, space="PSUM") as ps_p,
            tc.tile_pool(name="ps_s", bufs=4, space="PSUM") as ps_s,
            tc.tile_pool(name="ps_a", bufs=2, space="PSUM") as ps_a,
        ):
            # --- stationary weights + bias, loaded once ---
            wq_sb, wk_sb, wv_sb = [], [], []
            for lst, src, nm in ((wq_sb, wq, "wq"), (wk_sb, wk, "wk"),
                                 (wv_sb, wv, "wv")):
                for kk in range(KC):
                    t = wpool.tile([128, DL], fp16, tag=f"{nm}{kk}",
                                   name=f"{nm}{kk}")
                    nc.sync.dma_start(out=t[:], in_=src[ts(kk, 128), :])
                    lst.append(t)
            bq_sb = wpool.tile([128, OD], f32, tag="bq")
            nc.sync.dma_start(out=bq_sb[:], in_=bqt[:, :])

            state = {}  # per-quarter tiles: qt, kt, v

            def load_xt(q):
                tok0 = q * TOKQ
                xt_sb = []
                for kk in range(KC):
                    t = xtp.tile([128, TOKQ], fp16, tag="xt")
                    nc.sync.dma_start(
                        out=t[:], in_=xt[ts(kk, 128), tok0:tok0 + TOKQ])
                    xt_sb.append(t)
                state[q] = {"xt": xt_sb}

            def proj_units(q):
                """Yield 24 emission units: 16 q/k groups + 8 v groups."""
                st = state[q]
                xt_sb = st["xt"]
                qt_sb = [qtp.tile([128, TOKQ], fp16, tag="qt", name="qt")
                         for _ in range(OD)]
                kt_sb = [ktp.tile([128, TOKQ], fp16, tag="kt", name="kt")
                         for _ in range(OD)]
                v_sb = [vpool.tile([128, HL * VW], bf16, tag="v", name="v")
                        for _ in range(NBQ)]
                st["qt"], st["kt"], st["v"] = qt_sb, kt_sb, v_sb

                def qk_unit(which, od, tt):
                    def emit():
                        w_sb = wq_sb if which == "q" else wk_sb
                        dst = qt_sb if which == "q" else kt_sb
                        p = ps_p.tile([128, 512], f32, tag="pp", name="pp")
                        for kk in range(KC):
                            nc.tensor.matmul(
                                p[:], w_sb[kk][:, ts(od, 128)],
                                xt_sb[kk][:, ts(tt, 512)],
                                start=(kk == 0), stop=(kk == KC - 1))
                        if which == "q":
                            nc.scalar.activation(
                                dst[od][:, ts(tt, 512)], p[:], Act.Identity,
                                bias=bq_sb[:, od:od + 1], scale=1.0)
                        else:
                            nc.scalar.copy(dst[od][:, ts(tt, 512)], p[:])
                    return emit

                def v_unit(vt):
                    def emit():
                        p = ps_p.tile([128, 512], f32, tag="pp", name="pp")
                        for kk in range(KC):
                            nc.tensor.matmul(
                                p[:], xt_sb[kk][:, ts(vt, 128)], wv_sb[kk][:],
                                start=(kk == 0), stop=(kk == KC - 1))
                        vt_sb = v_sb[vt]
                        v3 = vt_sb[:].rearrange("p (h c) -> p h c", c=VW)
                        nc.vector.memset(v3[:, :, 64:66], 1.0)
                        nc.vector.tensor_copy(
                            v3[:, :, 0:64],
                            p[:].rearrange("p (h c) -> p h c", c=HD))
                    return emit

                units = []
                for od in range(OD):
                    units.append(qk_unit("q", od, 0))
                    units.append(qk_unit("q", od, 1))
                    units.append(qk_unit("k", od, 0))
                    units.append(qk_unit("k", od, 1))
                for vt in range(NBQ):
                    units.append(v_unit(vt))
                return units

            def attn_scores(q, bk):
                """Part 1: scores matmuls + batched EXP for one bucket."""
                st = state[q]
                qt_sb, kt_sb = st["qt"], st["kt"]
                col = ts(bk, BS)  # token slice within quarter
                se = ps_s.tile([128, 512], f32, tag="ps", name="ps_e")
                so = ps_s.tile([128, 512], f32, tag="ps", name="ps_o")
                # even heads first so the EXP of bank se can start while the
                # odd-head score matmuls still stream
                for h in (0, 2, 4, 6, 1, 3, 5, 7):
                    od, po = h // 2, (h % 2) * 64
                    bank = se if h % 2 == 0 else so
                    nc.tensor.matmul(
                        bank[:, ts(h // 2, 128)],
                        kt_sb[od][po:po + 64, col],
                        qt_sb[od][po:po + 64, col],
                        start=True, stop=True)
                ex_e = epool.tile([128, 512], bf16, tag="ex", name="ex_e")
                ex_o = epool.tile([128, 512], bf16, tag="ex", name="ex_o")
                nc.scalar.activation(ex_e[:], se[:], Act.Exp)
                nc.scalar.activation(ex_o[:], so[:], Act.Exp)
                st.setdefault("ex", {})[bk] = (ex_e, ex_o)

            def attn_out(q, bk):
                """Part 2: attended matmuls + normalize + residual + out."""
                st = state[q]
                v_sb = st["v"]
                ex_e, ex_o = st["ex"].pop(bk)
                tok0 = q * TOKQ
                xr = xrpool.tile([128, DL], fp16, tag="xres")
                nc.sync.dma_start(
                    out=xr[:],
                    in_=xres[tok0 + bk * BS:tok0 + (bk + 1) * BS, :])
                pe = ps_a.tile([128, HL // 2 * VW], f32, tag="pa", name="pa_e")
                po_ = ps_a.tile([128, HL // 2 * VW], f32, tag="pa", name="pa_o")
                for h in (0, 2, 4, 6, 1, 3, 5, 7):
                    ex = ex_e if h % 2 == 0 else ex_o
                    bank = pe if h % 2 == 0 else po_
                    slot = h // 2
                    nc.tensor.matmul(
                        bank[:, slot * VW:slot * VW + VW],
                        ex[:, ts(slot, 128)],
                        v_sb[bk][:, h * VW:(h + 1) * VW],
                        start=True, stop=True)
                yt = ypool.tile([128, DL], f32, tag="yt")
                for par, bank in ((0, pe), (1, po_)):
                    pav = bank[:].rearrange("p (h c) -> p h c", c=VW)
                    rc = rpool.tile([128, HL // 2], f32, tag="rc")
                    nc.vector.reciprocal(
                        rc[:].unsqueeze(2), pav[:, :, 64:65])
                    ytv = yt[:].rearrange(
                        "p (h two c) -> p h two c", two=2, c=HD)[:, :, par, :]
                    rcb = rc[:].unsqueeze(2).broadcast_to((128, HL // 2, HD))
                    nc.vector.tensor_tensor(
                        out=ytv, in0=pav[:, :, 0:HD], in1=rcb, op=Alu.mult)
                nc.vector.tensor_tensor(
                    out=yt[:], in0=yt[:], in1=xr[:], op=Alu.add)
                nc.sync.dma_start(
                    out=y[tok0 + bk * BS:tok0 + (bk + 1) * BS, :], in_=yt[:])

            # --- emission: proj(q) interleaved with pipelined attention of
            # quarter q-1 (attended lags scores by one bucket) ---
            load_xt(0)
            for q in range(NQ):
                if q + 1 < NQ:
                    load_xt(q + 1)
                units = proj_units(q)
                if q == 0:
                    for u in units:
                        u()
                else:
                    ui = 0
                    for bk in range(NBQ):
                        units[ui](); units[ui + 1]()
                        attn_scores(q - 1, bk)
                        units[ui + 2]()
                        ui += 3
                        if bk > 0:
                            attn_out(q - 1, bk - 1)
                    attn_out(q - 1, NBQ - 1)
                    del state[q - 1]
            for bk in range(NBQ):
                attn_scores(NQ - 1, bk)
                if bk > 0:
                    attn_out(NQ - 1, bk - 1)
            attn_out(NQ - 1, NBQ - 1)

    _built = nc
    return nc


def _prep_in_maps(x, Wq, bq, Wk, bk, Wv, bv):
    x = np.asarray(x, np.float32)
    Wq = np.asarray(Wq, np.float32)
    Wv = np.asarray(Wv, np.float32)
    Wk = np.asarray(Wk, np.float32)
    bq = np.asarray(bq, np.float32)
    bv = np.asarray(bv, np.float32)

    xt_b = [np.ascontiguousarray(x[b].T).astype(FP16) for b in range(B)]
    wq_g, wk_g, wv_g, bq_g = [], [], [], []
    for g in range(HG):
        sl = slice(g * DL, (g + 1) * DL)
        wq_g.append(np.ascontiguousarray(Wq[sl, :].T).astype(FP16))
        wk_g.append(np.ascontiguousarray(Wk[sl, :].T).astype(FP16))
        wv_g.append(np.ascontiguousarray(Wv[sl, :].T).astype(FP16))
        bq_g.append(np.ascontiguousarray(
            bq[sl].reshape(DL // 128, 128).T).astype(np.float32))

    in_maps = []
    for c in range(NCORES):
        b, g = c // HG, c % HG
        sl = slice(g * DL, (g + 1) * DL)
        xres = (x[b][:, sl] + bv[None, sl]).astype(FP16)
        in_maps.append({
            "xt": xt_b[b], "wq": wq_g[g], "wk": wk_g[g], "wv": wv_g[g],
            "bq": bq_g[g], "xres": np.ascontiguousarray(xres),
        })
    return in_maps


def _gather(results):
    out = np.empty((B, S, D), np.float32)
    for c, r in enumerate(results):
        b, g = c // HG, c % HG
        out[b, :, g * DL:(g + 1) * DL] = r["y"]
    return out


def _run(inputs, trace=False, trace_cores=None):
    nc = _build()
    from concourse.bass_utils import run_bass_kernel_spmd

    in_maps = _prep_in_maps(**inputs)
    res = run_bass_kernel_spmd(
        nc, in_maps, core_ids=list(range(NCORES)), trace=trace,
        trace_cores=trace_cores)
    return _gather(res.results), res


def kernel(**inputs):
    out, _ = _run(inputs, trace=False)
    return out


def kernel_traced(trace_cores=None, **inputs):
    """For test.py: returns (output, BassKernelResults with exec_time_ns)."""
    import types
    import trn_agent_boot.trn_boot as tb

    if "antenv.axon_hooks" not in sys.modules:
        hooks = types.ModuleType("antenv.axon_hooks")
        state = [None]
        hooks.set_axon_ntff_profile_hook = lambda h: state.__setitem__(0, h)
        hooks.get_axon_ntff_profile_hook = lambda: state[0]
        sys.modules["antenv.axon_hooks"] = hooks
        hooks.set_axon_ntff_profile_hook(
            tb._ntff_profile_via_ctypes("/opt/axon/libaxon_pjrt.so"))
    return _run(inputs, trace=True, trace_cores=trace_cores)


# revision 9
# speedup vs baseline: 1.6184x; 1.0614x over previous
"""Bucket (block-diagonal) attention layer for Trainium2, 8 NeuronCores SPMD.

Sharding: data-parallel over batch (4) x tensor-parallel over head groups (2).
Core c = b*2 + g handles batch b, global heads [g*8, g*8+8).

Per-core math (local out dim 512 = 8 heads x 64):
  qT[dl, t] = sum_k Wq[g*512+dl, k] * x[b, t, k]  (+ bq)   [transposed layout]
  kT[dl, t] = likewise (bk dropped: constant-per-row score shifts cancel in
              softmax -- only bq enters scores via bq . k_j)
  v[t, dl]  = natural layout (bf16), with a ones-column appended per head so
              the attended matmul also produces the softmax denominator.
  scoresT[kt, qt] = matmul(lhsT=kT_head, rhs=qT_head)      (K=64)
  expT = exp(scoresT) in bf16 (no max subtraction; logits sigma ~3.3)
  att[qt, 0:64], den[qt] = matmul(lhsT=expT, rhs=[v_head | ones])  (bf16)
  y = att / den + (x_slice + bv)   [residual + bv folded on host, fp16]

Perf structure vs v1 baseline:
 - all attention matmuls 16-bit (v1 ran them fp32 = 4 cycles/row on PE)
 - scores for 4 heads share one PSUM bank -> one batched EXP per [128,512]
 - attended for 4 heads share one bank -> batched reciprocal + strided
   broadcast normalize on DVE (v1: per-head ops)
 - projections of quarter q+1 are emission-interleaved with attention of
   quarter q so the PE stays dense (HAM stays warm) and softmax latency
   hides under projection matmuls.
"""

import json
import sys

import numpy as np

FP16 = np.float16

B, S, D = 4, 4096, 1024
H, NB = 16, 32
HG = 2            # head groups (tensor parallel over heads)
NCORES = B * HG   # 8
DL = D // HG      # 512 local output dims per core
HL = H // HG      # 8 local heads
HD = D // H       # 64 head dim
BS = S // NB      # 128 bucket size
KC = D // 128     # 8 contraction chunks
NQ = 4            # token quarters processed as pipeline phases
TOKQ = S // NQ    # 1024 tokens per quarter
NBQ = TOKQ // BS  # 8 buckets per quarter
OD = DL // 128    # 4 out-dim partition tiles for qT/kT
VW = 66           # per-head block width in v tiles: 64 data + 1 ones + 1 pad

_built = None     # cached (nc,) so repeated kernel() calls reuse the program


def _apply_waitfix():
    """This container's walrus accepts at most ONE sem wait per instruction.
    Post-process the BIR json: hoist extra waits onto injected wait-only
    EventSemaphore instructions just before the owning instruction."""
    import concourse.bass as bass

    if getattr(bass.Bass, "_waitfix_applied", False):
        return
    orig = bass.Bass.to_json_bytes

    def _split(m):
        n = 0
        for f in m["functions"]:
            for blk in f["blocks"]:
                out = []
                for inst in blk["instructions"]:
                    si = inst.get("sync_info")
                    if si and si.get("on_wait") and len(si["on_wait"]) > 1:
                        waits = si["on_wait"]
                        si["on_wait"] = waits[-1:]
                        for k, w in enumerate(waits[:-1]):
                            out.append({
                                "debug": inst.get("debug", 0),
                                "engine": inst["engine"],
                                "ins": [],
                                "outs": [],
                                "name": f"wfix{n}_{k}_{inst['name']}",
                                "opcode": "EventSemaphore",
                                "sync_info": {"on_update": [], "on_wait": [w]},
                            })
                        n += 1
                    out.append(inst)
                blk["instructions"] = out
        return n

    def patched(self):
        m = json.loads(orig(self))
        _split(m)
        return json.dumps(m).encode()

    bass.Bass.to_json_bytes = patched
    bass.Bass._waitfix_applied = True


def _build():
    global _built
    if _built is not None:
        return _built

    _apply_waitfix()
    import concourse.bass as bass
    import concourse.tile as tile
    from concourse import mybir
    from concourse.bass import ts

    f32 = mybir.dt.float32
    fp16 = mybir.dt.float16
    bf16 = mybir.dt.bfloat16
    Act = mybir.ActivationFunctionType
    Alu = mybir.AluOpType

    nc = bass.Bass()
    xt = nc.dram_tensor("xt", [D, S], fp16, kind="ExternalInput")
    wq = nc.dram_tensor("wq", [D, DL], fp16, kind="ExternalInput")
    wk = nc.dram_tensor("wk", [D, DL], fp16, kind="ExternalInput")
    wv = nc.dram_tensor("wv", [D, DL], fp16, kind="ExternalInput")
    bqt = nc.dram_tensor("bq", [128, OD], f32, kind="ExternalInput")
    xres = nc.dram_tensor("xres", [S, DL], fp16, kind="ExternalInput")
    y = nc.dram_tensor("y", [S, DL], f32, kind="ExternalOutput")

    with tile.TileContext(nc) as tc:
        with (
            tc.tile_pool(name="wpool", bufs=1) as wpool,
            tc.tile_pool(name="xtp", bufs=2) as xtp,
            tc.tile_pool(name="qtp", bufs=2 * OD) as qtp,
            tc.tile_pool(name="ktp", bufs=2 * OD) as ktp,
            tc.tile_pool(name="vp", bufs=2 * NBQ) as vpool,
            tc.tile_pool(name="ep", bufs=4) as epool,
            tc.tile_pool(name="yp", bufs=3) as ypool,
            tc.tile_pool(name="xrp", bufs=4) as xrpool,
            tc.tile_pool(name="rp", bufs=8) as rpool,
            tc.tile_pool(name="ps_p", bufs=2, space="PSUM") as ps_p,
            tc.tile_pool(name="ps_s", bufs=4, space="PSUM") as ps_s,
            tc.tile_pool(name="ps_a", bufs=2, space="PSUM") as ps_a,
        ):
            # --- stationary weights + bias, one big DMA per tensor ---
            # (each dma_start costs ~300ns of descriptor fan-out on the sync
            # sequencer; 25 small weight loads serialized the head)
            def wload(src, nm):
                t = wpool.tile([128, KC, DL], fp16, tag=nm, name=nm)
                nc.sync.dma_start(
                    out=t[:], in_=src[:, :].rearrange("(kk p) d -> p kk d",
                                                      p=128))
                return t

            wq_t = wload(wq, "wq")  # first: gates the first projections

            state = {}  # per-quarter tiles: qt, kt, v

            def load_xt(q, tt=None):
                """One big DMA per (quarter, token-half)."""
                tok0 = q * TOKQ
                if q not in state:
                    state[q] = {"xt": xtp.tile(
                        [128, KC, TOKQ], fp16, tag="xt", name="xt")}
                t = state[q]["xt"]
                for tth in (range(2) if tt is None else [tt]):
                    sl = slice(tok0 + tth * 512, tok0 + (tth + 1) * 512)
                    nc.sync.dma_start(
                        out=t[:, :, tth * 512:(tth + 1) * 512],
                        in_=xt[:, sl].rearrange("(kk p) t -> p kk t", p=128))

            load_xt(0, tt=0)
            wk_t = wload(wk, "wk")
            load_xt(0, tt=1)
            bq_sb = wpool.tile([128, OD], f32, tag="bq")
            nc.sync.dma_start(out=bq_sb[:], in_=bqt[:, :])
            wv_t = wload(wv, "wv")
            wq_sb = [wq_t[:, kk, :] for kk in range(KC)]
            wk_sb = [wk_t[:, kk, :] for kk in range(KC)]
            wv_sb = [wv_t[:, kk, :] for kk in range(KC)]

            def proj_units(q):
                """Yield 24 emission units: 16 q/k groups + 8 v groups."""
                st = state[q]
                xt_t = st["xt"]
                qt_sb = [qtp.tile([128, TOKQ], fp16, tag="qt", name="qt")
                         for _ in range(OD)]
                kt_sb = [ktp.tile([128, TOKQ], fp16, tag="kt", name="kt")
                         for _ in range(OD)]
                v_sb = [vpool.tile([128, HL * VW], bf16, tag="v", name="v")
                        for _ in range(NBQ)]
                st["qt"], st["kt"], st["v"] = qt_sb, kt_sb, v_sb

                def qk_unit(which, od, tt):
                    def emit():
                        w_t = wq_t if which == "q" else wk_t
                        dst = qt_sb if which == "q" else kt_sb
                        p = ps_p.tile([128, 512], f32, tag="pp", name="pp")
                        for kk in range(KC):
                            nc.tensor.matmul(
                                p[:], w_t[:, kk, ts(od, 128)],
                                xt_t[:, kk, ts(tt, 512)],
                                start=(kk == 0), stop=(kk == KC - 1))
                        if which == "q":
                            nc.scalar.activation(
                                dst[od][:, ts(tt, 512)], p[:], Act.Identity,
                                bias=bq_sb[:, od:od + 1], scale=1.0)
                        else:
                            nc.scalar.copy(dst[od][:, ts(tt, 512)], p[:])
                    return emit

                def v_unit(vt):
                    def emit():
                        p = ps_p.tile([128, 512], f32, tag="pp", name="pp")
                        for kk in range(KC):
                            nc.tensor.matmul(
                                p[:], xt_t[:, kk, ts(vt, 128)], wv_t[:, kk, :],
                                start=(kk == 0), stop=(kk == KC - 1))
                        vt_sb = v_sb[vt]
                        v3 = vt_sb[:].rearrange("p (h c) -> p h c", c=VW)
                        nc.vector.memset(v3[:, :, 64:66], 1.0)
                        nc.vector.tensor_copy(
                            v3[:, :, 0:64],
                            p[:].rearrange("p (h c) -> p h c", c=HD))
                    return emit

                # tt-major so quarter 0 can start on the first half of xt
                units = []
                for tt in range(2):
                    for od in range(OD):
                        units.append(qk_unit("q", od, tt))
                        units.append(qk_unit("k", od, tt))
                for vt in range(NBQ):
                    units.append(v_unit(vt))
                return units

            def attn_scores(q, bk):
                """Part 1: scores matmuls + batched EXP for one bucket."""
                st = state[q]
                qt_sb, kt_sb = st["qt"], st["kt"]
                col = ts(bk, BS)  # token slice within quarter
                se = ps_s.tile([128, 512], f32, tag="ps", name="ps_e")
                so = ps_s.tile([128, 512], f32, tag="ps", name="ps_o")
                # even heads first so the EXP of bank se can start while the
                # odd-head score matmuls still stream
                for h in (0, 2, 4, 6, 1, 3, 5, 7):
                    od, po = h // 2, (h % 2) * 64
                    bank = se if h % 2 == 0 else so
                    nc.tensor.matmul(
                        bank[:, ts(h // 2, 128)],
                        kt_sb[od][po:po + 64, col],
                        qt_sb[od][po:po + 64, col],
                        start=True, stop=True)
                ex_e = epool.tile([128, 512], bf16, tag="ex", name="ex_e")
                ex_o = epool.tile([128, 512], bf16, tag="ex", name="ex_o")
                nc.scalar.activation(ex_e[:], se[:], Act.Exp)
                nc.scalar.activation(ex_o[:], so[:], Act.Exp)
                st.setdefault("ex", {})[bk] = (ex_e, ex_o)

            def attn_out(q, bk):
                """Part 2: attended matmuls + normalize + residual + out."""
                st = state[q]
                v_sb = st["v"]
                ex_e, ex_o = st["ex"].pop(bk)
                tok0 = q * TOKQ
                xr = xrpool.tile([128, DL], fp16, tag="xres")
                nc.sync.dma_start(
                    out=xr[:],
                    in_=xres[tok0 + bk * BS:tok0 + (bk + 1) * BS, :])
                pe = ps_a.tile([128, HL // 2 * VW], f32, tag="pa", name="pa_e")
                po_ = ps_a.tile([128, HL // 2 * VW], f32, tag="pa", name="pa_o")
                for h in (0, 2, 4, 6, 1, 3, 5, 7):
                    ex = ex_e if h % 2 == 0 else ex_o
                    bank = pe if h % 2 == 0 else po_
                    slot = h // 2
                    nc.tensor.matmul(
                        bank[:, slot * VW:slot * VW + VW],
                        ex[:, ts(slot, 128)],
                        v_sb[bk][:, h * VW:(h + 1) * VW],
                        start=True, stop=True)
                yt = ypool.tile([128, DL], f32, tag="yt")
                for par, bank in ((0, pe), (1, po_)):
                    pav = bank[:].rearrange("p (h c) -> p h c", c=VW)
                    rc = rpool.tile([128, HL // 2], f32, tag="rc")
                    nc.vector.reciprocal(
                        rc[:].unsqueeze(2), pav[:, :, 64:65])
                    ytv = yt[:].rearrange(
                        "p (h two c) -> p h two c", two=2, c=HD)[:, :, par, :]
                    rcb = rc[:].unsqueeze(2).broadcast_to((128, HL // 2, HD))
                    nc.vector.tensor_tensor(
                        out=ytv, in0=pav[:, :, 0:HD], in1=rcb, op=Alu.mult)
                nc.vector.tensor_tensor(
                    out=yt[:], in0=yt[:], in1=xr[:], op=Alu.add)
                nc.sync.dma_start(
                    out=y[tok0 + bk * BS:tok0 + (bk + 1) * BS, :], in_=yt[:])

            # --- emission: proj(q) interleaved with pipelined attention of
            # quarter q-1 (attended lags scores by one bucket); next
            # quarter's xt prefetch fires after the qk units ---
            for q in range(NQ):
                units = proj_units(q)
                if q == 0:
                    for ui, u in enumerate(units):
                        u()
                        if ui == 15 and q + 1 < NQ:
                            load_xt(q + 1)
                else:
                    ui = 0
                    for bk in range(NBQ):
                        units[ui](); units[ui + 1]()
                        attn_scores(q - 1, bk)
                        units[ui + 2]()
                        ui += 3
                        if ui == 18 and q + 1 < NQ:
                            load_xt(q + 1)
                        if bk > 0:
                            attn_out(q - 1, bk - 1)
                    attn_out(q - 1, NBQ - 1)
                    del state[q - 1]["qt"], state[q - 1]["kt"]
            for bk in range(NBQ):
                attn_scores(NQ - 1, bk)
                if bk > 0:
                    attn_out(NQ - 1, bk - 1)
            attn_out(NQ - 1, NBQ - 1)

    _built = nc
    return nc


def _prep_in_maps(x, Wq, bq, Wk, bk, Wv, bv):
    x = np.asarray(x, np.float32)
    Wq = np.asarray(Wq, np.float32)
    Wv = np.asarray(Wv, np.float32)
    Wk = np.asarray(Wk, np.float32)
    bq = np.asarray(bq, np.float32)
    bv = np.asarray(bv, np.float32)

    xt_b = [np.ascontiguousarray(x[b].T).astype(FP16) for b in range(B)]
    wq_g, wk_g, wv_g, bq_g = [], [], [], []
    for g in range(HG):
        sl = slice(g * DL, (g + 1) * DL)
        wq_g.append(np.ascontiguousarray(Wq[sl, :].T).astype(FP16))
        wk_g.append(np.ascontiguousarray(Wk[sl, :].T).astype(FP16))
        wv_g.append(np.ascontiguousarray(Wv[sl, :].T).astype(FP16))
        bq_g.append(np.ascontiguousarray(
            bq[sl].reshape(DL // 128, 128).T).astype(np.float32))

    in_maps = []
    for c in range(NCORES):
        b, g = c // HG, c % HG
        sl = slice(g * DL, (g + 1) * DL)
        xres = (x[b][:, sl] + bv[None, sl]).astype(FP16)
        in_maps.append({
            "xt": xt_b[b], "wq": wq_g[g], "wk": wk_g[g], "wv": wv_g[g],
            "bq": bq_g[g], "xres": np.ascontiguousarray(xres),
        })
    return in_maps


def _gather(results):
    out = np.empty((B, S, D), np.float32)
    for c, r in enumerate(results):
        b, g = c // HG, c % HG
        out[b, :, g * DL:(g + 1) * DL] = r["y"]
    return out


def _run(inputs, trace=False, trace_cores=None):
    nc = _build()
    from concourse.bass_utils import run_bass_kernel_spmd

    in_maps = _prep_in_maps(**inputs)
    res = run_bass_kernel_spmd(
        nc, in_maps, core_ids=list(range(NCORES)), trace=trace,
        trace_cores=trace_cores)
    return _gather(res.results), res


def kernel(**inputs):
    out, _ = _run(inputs, trace=False)
    return out


def kernel_traced(trace_cores=None, **inputs):
    """For test.py: returns (output, BassKernelResults with exec_time_ns)."""
    import types
    import trn_agent_boot.trn_boot as tb

    if "antenv.axon_hooks" not in sys.modules:
        hooks = types.ModuleType("antenv.axon_hooks")
        state = [None]
        hooks.set_axon_ntff_profile_hook = lambda h: state.__setitem__(0, h)
        hooks.get_axon_ntff_profile_hook = lambda: state[0]
        sys.modules["antenv.axon_hooks"] = hooks
        hooks.set_axon_ntff_profile_hook(
            tb._ntff_profile_via_ctypes("/opt/axon/libaxon_pjrt.so"))
    return _run(inputs, trace=True, trace_cores=trace_cores)


# revision 10
# speedup vs baseline: 1.7067x; 1.0546x over previous
"""Bucket (block-diagonal) attention layer for Trainium2, 8 NeuronCores SPMD.

Sharding: data-parallel over batch (4) x tensor-parallel over head groups (2).
Core c = b*2 + g handles batch b, global heads [g*8, g*8+8).

Per-core math (local out dim 512 = 8 heads x 64):
  qT[dl, t] = sum_k Wq[g*512+dl, k] * x[b, t, k]  (+ bq)   [transposed layout]
  kT[dl, t] = likewise (bk dropped: constant-per-row score shifts cancel in
              softmax -- only bq enters scores via bq . k_j)
  v[t, dl]  = natural layout (bf16), with a ones-column appended per head so
              the attended matmul also produces the softmax denominator.
  scoresT[kt, qt] = matmul(lhsT=kT_head, rhs=qT_head)      (K=64)
  expT = exp(scoresT) in bf16 (no max subtraction; logits sigma ~3.3)
  att[qt, 0:64], den[qt] = matmul(lhsT=expT, rhs=[v_head | ones])  (bf16)
  y = att / den + (x_slice + bv)   [residual + bv folded on host, fp16]

Perf structure vs v1 baseline:
 - all attention matmuls 16-bit (v1 ran them fp32 = 4 cycles/row on PE)
 - scores for 4 heads share one PSUM bank -> one batched EXP per [128,512]
 - attended for 4 heads share one bank -> batched reciprocal + strided
   broadcast normalize on DVE (v1: per-head ops)
 - projections of quarter q+1 are emission-interleaved with attention of
   quarter q so the PE stays dense (HAM stays warm) and softmax latency
   hides under projection matmuls.
"""

import json
import sys

import numpy as np

FP16 = np.float16

B, S, D = 4, 4096, 1024
H, NB = 16, 32
HG = 2            # head groups (tensor parallel over heads)
NCORES = B * HG   # 8
DL = D // HG      # 512 local output dims per core
HL = H // HG      # 8 local heads
HD = D // H       # 64 head dim
BS = S // NB      # 128 bucket size
KC = D // 128     # 8 contraction chunks
NQ = 4            # token quarters processed as pipeline phases
TOKQ = S // NQ    # 1024 tokens per quarter
NBQ = TOKQ // BS  # 8 buckets per quarter
OD = DL // 128    # 4 out-dim partition tiles for qT/kT
VW = 66           # per-head block width in v tiles: 64 data + 1 ones + 1 pad

_built = None     # cached (nc,) so repeated kernel() calls reuse the program


def _apply_waitfix():
    """This container's walrus accepts at most ONE sem wait per instruction.
    Post-process the BIR json: hoist extra waits onto injected wait-only
    EventSemaphore instructions just before the owning instruction."""
    import concourse.bass as bass

    if getattr(bass.Bass, "_waitfix_applied", False):
        return
    orig = bass.Bass.to_json_bytes

    def _split(m):
        n = 0
        for f in m["functions"]:
            for blk in f["blocks"]:
                out = []
                for inst in blk["instructions"]:
                    si = inst.get("sync_info")
                    if si and si.get("on_wait") and len(si["on_wait"]) > 1:
                        waits = si["on_wait"]
                        si["on_wait"] = waits[-1:]
                        for k, w in enumerate(waits[:-1]):
                            out.append({
                                "debug": inst.get("debug", 0),
                                "engine": inst["engine"],
                                "ins": [],
                                "outs": [],
                                "name": f"wfix{n}_{k}_{inst['name']}",
                                "opcode": "EventSemaphore",
                                "sync_info": {"on_update": [], "on_wait": [w]},
                            })
                        n += 1
                    out.append(inst)
                blk["instructions"] = out
        return n

    def patched(self):
        m = json.loads(orig(self))
        _split(m)
        return json.dumps(m).encode()

    bass.Bass.to_json_bytes = patched
    bass.Bass._waitfix_applied = True


def _build():
    global _built
    if _built is not None:
        return _built

    _apply_waitfix()
    import concourse.bass as bass
    import concourse.tile as tile
    from concourse import mybir
    from concourse.bass import ts

    f32 = mybir.dt.float32
    fp16 = mybir.dt.float16
    bf16 = mybir.dt.bfloat16
    Act = mybir.ActivationFunctionType
    Alu = mybir.AluOpType

    nc = bass.Bass()
    xt = nc.dram_tensor("xt", [D, S], fp16, kind="ExternalInput")
    wq = nc.dram_tensor("wq", [D, DL], fp16, kind="ExternalInput")
    wk = nc.dram_tensor("wk", [D, DL], fp16, kind="ExternalInput")
    wv = nc.dram_tensor("wv", [D, DL], fp16, kind="ExternalInput")
    bqt = nc.dram_tensor("bq", [128, OD], f32, kind="ExternalInput")
    xres = nc.dram_tensor("xres", [S, DL], fp16, kind="ExternalInput")
    y = nc.dram_tensor("y", [S, DL], f32, kind="ExternalOutput")

    with tile.TileContext(nc) as tc:
        with (
            tc.tile_pool(name="wpool", bufs=1) as wpool,
            tc.tile_pool(name="xtp", bufs=2) as xtp,
            tc.tile_pool(name="qtp", bufs=2 * OD) as qtp,
            tc.tile_pool(name="ktp", bufs=2 * OD) as ktp,
            tc.tile_pool(name="vp", bufs=2 * NBQ) as vpool,
            tc.tile_pool(name="ep", bufs=4) as epool,
            tc.tile_pool(name="yp", bufs=3) as ypool,
            tc.tile_pool(name="xrp", bufs=4) as xrpool,
            tc.tile_pool(name="rp", bufs=8) as rpool,
            tc.tile_pool(name="ps_p", bufs=2, space="PSUM") as ps_p,
            tc.tile_pool(name="ps_s", bufs=4, space="PSUM") as ps_s,
            tc.tile_pool(name="ps_a", bufs=2, space="PSUM") as ps_a,
        ):
            # --- stationary weights + bias, one big DMA per tensor ---
            # (each dma_start costs ~300ns of descriptor fan-out on the sync
            # sequencer; 25 small weight loads serialized the head)
            def wload(src, nm):
                t = wpool.tile([128, KC, DL], fp16, tag=nm, name=nm)
                nc.sync.dma_start(
                    out=t[:], in_=src[:, :].rearrange("(kk p) d -> p kk d",
                                                      p=128))
                return t

            wq_t = wload(wq, "wq")  # first: gates the first projections

            state = {}  # per-quarter tiles: qt, kt, v

            def load_xt(q, tt=None):
                """One big DMA per (quarter, token-half)."""
                tok0 = q * TOKQ
                if q not in state:
                    state[q] = {"xt": xtp.tile(
                        [128, KC, TOKQ], fp16, tag="xt", name="xt")}
                t = state[q]["xt"]
                for tth in (range(2) if tt is None else [tt]):
                    sl = slice(tok0 + tth * 512, tok0 + (tth + 1) * 512)
                    nc.sync.dma_start(
                        out=t[:, :, tth * 512:(tth + 1) * 512],
                        in_=xt[:, sl].rearrange("(kk p) t -> p kk t", p=128))

            load_xt(0, tt=0)
            wk_t = wload(wk, "wk")
            load_xt(0, tt=1)
            bq_sb = wpool.tile([128, OD], f32, tag="bq")
            nc.sync.dma_start(out=bq_sb[:], in_=bqt[:, :])
            wv_t = wload(wv, "wv")
            wq_sb = [wq_t[:, kk, :] for kk in range(KC)]
            wk_sb = [wk_t[:, kk, :] for kk in range(KC)]
            wv_sb = [wv_t[:, kk, :] for kk in range(KC)]

            def proj_units(q):
                """Yield 24 emission units: 16 q/k groups + 8 v groups."""
                st = state[q]
                xt_t = st["xt"]
                qt_sb = [qtp.tile([128, TOKQ], fp16, tag="qt", name="qt")
                         for _ in range(OD)]
                kt_sb = [ktp.tile([128, TOKQ], fp16, tag="kt", name="kt")
                         for _ in range(OD)]
                v_sb = [vpool.tile([128, HL * VW], bf16, tag="v", name="v")
                        for _ in range(NBQ)]
                st["qt"], st["kt"], st["v"] = qt_sb, kt_sb, v_sb

                def qk_unit(which, od, tt):
                    def emit():
                        w_t = wq_t if which == "q" else wk_t
                        dst = qt_sb if which == "q" else kt_sb
                        p = ps_p.tile([128, 512], f32, tag="pp", name="pp")
                        for kk in range(KC):
                            nc.tensor.matmul(
                                p[:], w_t[:, kk, ts(od, 128)],
                                xt_t[:, kk, ts(tt, 512)],
                                start=(kk == 0), stop=(kk == KC - 1))
                        if which == "q":
                            nc.scalar.activation(
                                dst[od][:, ts(tt, 512)], p[:], Act.Identity,
                                bias=bq_sb[:, od:od + 1], scale=1.0)
                        else:
                            nc.scalar.copy(dst[od][:, ts(tt, 512)], p[:])
                    return emit

                def v_unit(vt):
                    def emit():
                        p = ps_p.tile([128, 512], f32, tag="pp", name="pp")
                        for kk in range(KC):
                            nc.tensor.matmul(
                                p[:], xt_t[:, kk, ts(vt, 128)], wv_t[:, kk, :],
                                start=(kk == 0), stop=(kk == KC - 1))
                        vt_sb = v_sb[vt]
                        v3 = vt_sb[:].rearrange("p (h c) -> p h c", c=VW)
                        nc.vector.memset(v3[:, :, 64:66], 1.0)
                        nc.vector.tensor_copy(
                            v3[:, :, 0:64],
                            p[:].rearrange("p (h c) -> p h c", c=HD))
                    return emit

                # tt-major so quarter 0 can start on the first half of xt
                units = []
                for tt in range(2):
                    for od in range(OD):
                        units.append(qk_unit("q", od, tt))
                        units.append(qk_unit("k", od, tt))
                for vt in range(NBQ):
                    units.append(v_unit(vt))
                return units

            def attn_scores(q, bk):
                """Part 1: scores matmuls + batched EXP for one bucket."""
                st = state[q]
                qt_sb, kt_sb = st["qt"], st["kt"]
                col = ts(bk, BS)  # token slice within quarter
                se = ps_s.tile([128, 512], f32, tag="ps", name="ps_e")
                so = ps_s.tile([128, 512], f32, tag="ps", name="ps_o")
                # even heads first so the EXP of bank se can start while the
                # odd-head score matmuls still stream
                for h in (0, 2, 4, 6, 1, 3, 5, 7):
                    od, po = h // 2, (h % 2) * 64
                    bank = se if h % 2 == 0 else so
                    nc.tensor.matmul(
                        bank[:, ts(h // 2, 128)],
                        kt_sb[od][po:po + 64, col],
                        qt_sb[od][po:po + 64, col],
                        start=True, stop=True)
                ex_e = epool.tile([128, 512], bf16, tag="ex", name="ex_e")
                ex_o = epool.tile([128, 512], bf16, tag="ex", name="ex_o")
                nc.scalar.activation(ex_e[:], se[:], Act.Exp)
                nc.scalar.activation(ex_o[:], so[:], Act.Exp)
                st.setdefault("ex", {})[bk] = (ex_e, ex_o)

            def attn_out(q, bk):
                """Part 2: attended matmuls + normalize + residual + out."""
                st = state[q]
                v_sb = st["v"]
                ex_e, ex_o = st["ex"].pop(bk)
                tok0 = q * TOKQ
                xr = xrpool.tile([128, DL], fp16, tag="xres")
                nc.sync.dma_start(
                    out=xr[:],
                    in_=xres[tok0 + bk * BS:tok0 + (bk + 1) * BS, :])
                pe = ps_a.tile([128, HL // 2 * VW], f32, tag="pa", name="pa_e")
                po_ = ps_a.tile([128, HL // 2 * VW], f32, tag="pa", name="pa_o")
                for h in (0, 2, 4, 6, 1, 3, 5, 7):
                    ex = ex_e if h % 2 == 0 else ex_o
                    bank = pe if h % 2 == 0 else po_
                    slot = h // 2
                    nc.tensor.matmul(
                        bank[:, slot * VW:slot * VW + VW],
                        ex[:, ts(slot, 128)],
                        v_sb[bk][:, h * VW:(h + 1) * VW],
                        start=True, stop=True)
                yt = ypool.tile([128, DL], f32, tag="yt")
                for par, bank in ((0, pe), (1, po_)):
                    pav = bank[:].rearrange("p (h c) -> p h c", c=VW)
                    rc = rpool.tile([128, HL // 2], f32, tag="rc")
                    nc.vector.reciprocal(
                        rc[:].unsqueeze(2), pav[:, :, 64:65])
                    ytv = yt[:].rearrange(
                        "p (h two c) -> p h two c", two=2, c=HD)[:, :, par, :]
                    rcb = rc[:].unsqueeze(2).broadcast_to((128, HL // 2, HD))
                    nc.vector.tensor_tensor(
                        out=ytv, in0=pav[:, :, 0:HD], in1=rcb, op=Alu.mult)
                nc.vector.tensor_tensor(
                    out=yt[:], in0=yt[:], in1=xr[:], op=Alu.add)
                nc.sync.dma_start(
                    out=y[tok0 + bk * BS:tok0 + (bk + 1) * BS, :], in_=yt[:])

            # --- emission: per quarter, 16 q/k units then for each bucket
            # [v-unit, scores, attended(bk-1)] -- the EXP latency of bucket
            # bk hides under the v projection of bucket bk+1.  The last
            # bucket's attended spills into the next quarter's first unit.
            pending = None
            for q in range(NQ):
                units = proj_units(q)
                for i in range(2 * OD * 2):
                    units[i]()
                    if i == 0 and pending is not None:
                        attn_out(*pending)
                        pending = None
                    if i == 7 and q + 1 < NQ:
                        load_xt(q + 1)
                for bk in range(NBQ):
                    units[16 + bk]()
                    attn_scores(q, bk)
                    if bk > 0:
                        attn_out(q, bk - 1)
                pending = (q, NBQ - 1)
            attn_out(*pending)

    _built = nc
    return nc


def _prep_in_maps(x, Wq, bq, Wk, bk, Wv, bv):
    x = np.asarray(x, np.float32)
    Wq = np.asarray(Wq, np.float32)
    Wv = np.asarray(Wv, np.float32)
    Wk = np.asarray(Wk, np.float32)
    bq = np.asarray(bq, np.float32)
    bv = np.asarray(bv, np.float32)

    xt_b = [np.ascontiguousarray(x[b].T).astype(FP16) for b in range(B)]
    wq_g, wk_g, wv_g, bq_g = [], [], [], []
    for g in range(HG):
        sl = slice(g * DL, (g + 1) * DL)
        wq_g.append(np.ascontiguousarray(Wq[sl, :].T).astype(FP16))
        wk_g.append(np.ascontiguousarray(Wk[sl, :].T).astype(FP16))
        wv_g.append(np.ascontiguousarray(Wv[sl, :].T).astype(FP16))
        bq_g.append(np.ascontiguousarray(
            bq[sl].reshape(DL // 128, 128).T).astype(np.float32))

    in_maps = []
    for c in range(NCORES):
        b, g = c // HG, c % HG
        sl = slice(g * DL, (g + 1) * DL)
        xres = (x[b][:, sl] + bv[None, sl]).astype(FP16)
        in_maps.append({
            "xt": xt_b[b], "wq": wq_g[g], "wk": wk_g[g], "wv": wv_g[g],
            "bq": bq_g[g], "xres": np.ascontiguousarray(xres),
        })
    return in_maps


def _gather(results):
    out = np.empty((B, S, D), np.float32)
    for c, r in enumerate(results):
        b, g = c // HG, c % HG
        out[b, :, g * DL:(g + 1) * DL] = r["y"]
    return out


def _run(inputs, trace=False, trace_cores=None):
    nc = _build()
    from concourse.bass_utils import run_bass_kernel_spmd

    in_maps = _prep_in_maps(**inputs)
    res = run_bass_kernel_spmd(
        nc, in_maps, core_ids=list(range(NCORES)), trace=trace,
        trace_cores=trace_cores)
    return _gather(res.results), res


def kernel(**inputs):
    out, _ = _run(inputs, trace=False)
    return out


def kernel_traced(trace_cores=None, **inputs):
    """For test.py: returns (output, BassKernelResults with exec_time_ns)."""
    import types
    import trn_agent_boot.trn_boot as tb

    if "antenv.axon_hooks" not in sys.modules:
        hooks = types.ModuleType("antenv.axon_hooks")
        state = [None]
        hooks.set_axon_ntff_profile_hook = lambda h: state.__setitem__(0, h)
        hooks.get_axon_ntff_profile_hook = lambda: state[0]
        sys.modules["antenv.axon_hooks"] = hooks
        hooks.set_axon_ntff_profile_hook(
            tb._ntff_profile_via_ctypes("/opt/axon/libaxon_pjrt.so"))
    return _run(inputs, trace=True, trace_cores=trace_cores)
